# revision 50
# baseline (speedup 1.0000x reference)
"""Trainium2 Bass kernel for nn_AttractorCore (all-pairs N-body + MLP dynamics).

Self-contained: hardcodes shapes/sharding. Accepts FULL inputs, returns FULL output.
Distribution: data-parallel over bodies. M=4136 bodies padded to 4224 = 33*128;
each of 8 cores owns 528 rows. Per iteration each core computes its [528 x 4224]
tile of the interaction matrix (f16 matmuls; d^2 assembled by preloading
sq_j + sq_my into PSUM on GpSimd and accumulating -2*r_j.r_my on top, so the
old [2,128]-stationary sq matmuls are gone), takes d^-1 via one wide ACT per
j-tile, cubes/masks on DVE, and aggregates sum(w*r_j) plus sum(w) into spare
PSUM regions. PSUM is organized as [128,528] two-bank pairs with asymmetric
512/16 matmul splits so every ACT/preload is a single wide op. The j-major
position layout is rebuilt with PE transposes; per-flight DVE square+reduce
produces the j-major squared norms locally, so the AllGather ships positions
only. Final projection/stats epilogue on host.
"""
import os
import sys
import numpy as np
import ml_dtypes

sys.path.insert(0, "/opt/trn_rl_repo")

# ---- problem constants (hardcoded) ----
B, N, D, K = 4, 1024, 128, 10
G, DAMP, DT, ITERS, EPS, MAXNORM, HID = 1e-3, 0.95, 0.1, 50, 1e-6, 10.0, 64
M0 = B * N + B * K          # 4136 real bodies
NT = 33                     # j tiles of 128
MP = NT * 128               # 4224 padded bodies
NC = 8                      # cores
R = MP // NC                # 528 rows per core
G2 = G * DT * 0.1           # gravity * dt * uniform body mass (folded)
SA, SB = 512, 16            # asymmetric my split (PSUM bank containment)
PA, PB = 496, 32            # sp row split (bank tails)
H0, H1 = 256, R             # update-phase halves [0:256) / [256:528)
EPS2 = 1e-3                 # d^2 floor: keeps self-pair 1/d^3 finite (their
                            # force contribution cancels in f2p - sp*r)
# force-phase j-tile processing order: tiles fully covered by every core's
# first-half columns [0:256) come first so they can start while the
# second-half AllGather is still in flight. Slot s stores tile ORD[s].
ORD = [0, 1, 5, 9, 13, 17, 21, 25, 29,
       2, 3, 4, 6, 7, 8, 10, 11, 12, 14, 15, 16, 18, 19, 20,
       22, 23, 24, 26, 27, 28, 30, 31, 32]

_PROGRAM_CACHE = {}


def _build_program(iters):
    import concourse.bacc as bacc
    import concourse.mybir as mybir
    import concourse.tile as tile

    dt = mybir.dt
    F32, F16, U16 = dt.float32, dt.float16, dt.uint16
    AF = mybir.ActivationFunctionType
    OP = mybir.AluOpType

    nc = bacc.Bacc("TRN2", target_bir_lowering=False, debug=False, num_devices=NC)

    def din(name, shape, dtype):
        return nc.dram_tensor(name, shape, dtype, kind="ExternalInput").ap()

    i_rT0 = din("rT0", [128, R], F32)
    i_vT0 = din("vT0", [128, R], F32)
    i_rbT0 = din("rbT0", [128, MP], F16)
    i_rfin0 = din("rfin0", [128, R], F16)
    i_sqrow0 = din("sqrow0", [1, R], F32)     # |r0|^2 per my column
    i_maskU0 = din("maskU0", [128, NT * R], U16)
    i_mnrow = din("mnrow", [1, R], F32)       # max-norm per my column (1e30 pads)
    i_mn2row = din("mn2row", [1, R], F32)     # cap for sq
    i_id128 = din("id128", [128, 128], F16)
    i_bw1a = din("bw1a", [128, HID], F16)
    i_bw1b = din("bw1b", [128, HID], F16)
    i_bw2 = din("bw2", [HID, HID], F16)
    i_bw3 = din("bw3", [HID, 128], F16)
    i_pw1a = din("pw1a", [128, 128], F16)
    i_pw1b = din("pw1b", [128, 128], F16)
    i_pw1Ta = din("pw1Ta", [128, 128], F16)
    i_pw1Tb = din("pw1Tb", [128, 128], F16)
    i_kw1a = din("kw1a", [128, 128], F16)
    i_kw1b = din("kw1b", [128, 128], F16)
    i_kw1Ta = din("kw1Ta", [128, 128], F16)
    i_kw1Tb = din("kw1Tb", [128, 128], F16)
    i_bb1c = din("bb1c", [HID, 1], F32)
    i_bb2c = din("bb2c", [HID, 1], F32)
    i_bb3c = din("bb3c", [128, 1], F32)
    i_pb1c = din("pb1c", [128, 2], F32)       # col q
    i_kb1c = din("kb1c", [128, 2], F32)
    i_pw2c = din("pw2c", [128, 2], F32)
    i_pw2cn = din("pw2cn", [128, 2], F32)
    i_kw2c = din("kw2c", [128, 2], F32)
    i_kw2cn = din("kw2cn", [128, 2], F32)

    o_r = nc.dram_tensor("r_out", [128, R], F32, kind="ExternalOutput").ap()

    with tile.TileContext(nc) as tc:
        with (
            tc.tile_pool(name="cpool", bufs=1) as cpool,
            tc.tile_pool(name="wpool", bufs=1) as wpool,
            tc.tile_pool(name="d2pool", bufs=2, space="PSUM") as d2pool,
            tc.tile_pool(name="fpool", bufs=1, space="PSUM") as fpool,
            tc.tile_pool(name="spbpool", bufs=1, space="PSUM") as spbpool,
            tc.tile_pool(name="rnpool", bufs=1, space="PSUM") as rnpool,
            tc.tile_pool(name="dpool", bufs=3, space="DRAM") as dpool,
        ):
            # ---- persistent tiles ----
            rT = cpool.tile([128, R], F32, tag="rT")
            vT = cpool.tile([128, R], F32, tag="vT")
            rbT = cpool.tile([128, MP], F16, tag="rbT")
            RN = cpool.tile([128, MP], F16, tag="RN")       # j-major positions
            Y = cpool.tile([128, NT * R], F16, tag="Y")     # 1/d
            U = cpool.tile([128, NT * R], F16, tag="U")     # 1/d^3
            maskU = cpool.tile([128, NT * R], U16, tag="maskU")
            rfinb = cpool.tile([128, R], F16, tag="rfinb")
            rfneg = cpool.tile([128, R], F16, tag="rfneg")  # -2 * rfinb
            sqmyf = cpool.tile([1, R], F16, tag="sqmyf")    # |r_my|^2 row
            sqmyb = cpool.tile([128, R], F32, tag="sqmyb")  # bcast |r_my|^2
            sqrow = cpool.tile([1, R], F32, tag="sqrow")
            sqj = cpool.tile([128, NT], F32, tag="sqj")     # j-major |r_j|^2
            ones_row = cpool.tile([1, 128], F16, tag="ones_row")
            mnrow = cpool.tile([1, R], F32, tag="mnrow")
            mn2row = cpool.tile([1, R], F32, tag="mn2row")
            id128 = cpool.tile([128, 128], F16, tag="id128")
            ones_col = cpool.tile([128, 1], F16, tag="ones_col")
            bw1a = cpool.tile([128, HID], F16, tag="bw1a")
            bw1b = cpool.tile([128, HID], F16, tag="bw1b")
            bw2 = cpool.tile([HID, HID], F16, tag="bw2")
            bw3 = cpool.tile([HID, 128], F16, tag="bw3")
            pw1a = cpool.tile([128, 128], F16, tag="pw1a")
            pw1b = cpool.tile([128, 128], F16, tag="pw1b")
            pw1Ta = cpool.tile([128, 128], F16, tag="pw1Ta")
            pw1Tb = cpool.tile([128, 128], F16, tag="pw1Tb")
            kw1a = cpool.tile([128, 128], F16, tag="kw1a")
            kw1b = cpool.tile([128, 128], F16, tag="kw1b")
            kw1Ta = cpool.tile([128, 128], F16, tag="kw1Ta")
            kw1Tb = cpool.tile([128, 128], F16, tag="kw1Tb")
            bb1c = cpool.tile([HID, 1], F32, tag="bb1c")
            bb2c = cpool.tile([HID, 1], F32, tag="bb2c")
            bb3c = cpool.tile([128, 1], F32, tag="bb3c")
            pb1c = cpool.tile([128, 2], F32, tag="pb1c")
            kb1c = cpool.tile([128, 2], F32, tag="kb1c")
            pw2c = cpool.tile([128, 2], F32, tag="pw2c")
            pw2cn = cpool.tile([128, 2], F32, tag="pw2cn")
            kw2c = cpool.tile([128, 2], F32, tag="kw2c")
            kw2cn = cpool.tile([128, 2], F32, tag="kw2cn")

            dma = nc.sync.dma_start
            mm = nc.tensor.matmul
            act = nc.scalar.activation
            V = nc.vector
            GP = nc.gpsimd

            # ---- one-time loads ----
            for dst, src in [
                (rT, i_rT0), (vT, i_vT0), (rbT, i_rbT0), (rfinb, i_rfin0),
                (sqrow, i_sqrow0), (maskU, i_maskU0),
                (mnrow, i_mnrow), (mn2row, i_mn2row), (id128, i_id128),
                (bw1a, i_bw1a), (bw1b, i_bw1b), (bw2, i_bw2), (bw3, i_bw3),
                (pw1a, i_pw1a), (pw1b, i_pw1b), (pw1Ta, i_pw1Ta), (pw1Tb, i_pw1Tb),
                (kw1a, i_kw1a), (kw1b, i_kw1b), (kw1Ta, i_kw1Ta), (kw1Tb, i_kw1Tb),
                (bb1c, i_bb1c), (bb2c, i_bb2c), (bb3c, i_bb3c),
                (pb1c, i_pb1c), (kb1c, i_kb1c),
                (pw2c, i_pw2c), (pw2cn, i_pw2cn), (kw2c, i_kw2c), (kw2cn, i_kw2cn),
            ]:
                dma(dst[:], src)
            nc.gpsimd.memset(ones_col[:], 1.0)
            nc.gpsimd.memset(ones_row[:], 1.0)
            # initial derived state
            V.tensor_scalar_mul(rfneg[:], rfinb[:], -2.0)
            V.tensor_copy(sqmyf[:], sqrow[:])
            GP.partition_broadcast(sqmyb[:], sqrow[:])

            LAG = 6     # j-tiles between d2/act emission and f2p consumption
            GV = 4      # j-tiles per DVE mask/square/cube group

            def emit_flight(f):
                # PE-transpose 4 j-tiles of rbT into alternating halves of
                # the rnp PSUM bank (slot order): flight f+1's transposes
                # overlap flight f's Scalar PSUM->SBUF copy instead of
                # WAR-stalling on a single staging region.
                rnp = rnpool.tile([128, 1024], F16, tag="rnp")
                lo = 4 * f
                hi = min(lo + 4, NT)
                base = (f % 2) * 512
                for s in range(lo, hi):
                    t = ORD[s]
                    nc.tensor.transpose(
                        rnp[:, base + (s - lo) * 128:base + (s - lo + 1) * 128],
                        rbT[:, t * 128:(t + 1) * 128], id128[:])
                w = (hi - lo) * 128
                act(RN[:, lo * 128:lo * 128 + w], rnp[:, base:base + w],
                    AF.Copy)
                rnsq = wpool.tile([128, 1024], F16, tag="rnsq")
                V.tensor_tensor(rnsq[:, 0:w], RN[:, lo * 128:lo * 128 + w],
                                RN[:, lo * 128:lo * 128 + w], op=OP.mult)
                V.tensor_reduce(
                    sqj[:, lo:hi],
                    rnsq[:, 0:w].rearrange("p (t c) -> p t c", c=128),
                    axis=mybir.AxisListType.X, op=OP.add)

            FL_AT = {0: 0, 2: 1, 6: 2, 10: 3, 14: 4,
                     18: 5, 22: 6, 26: 7, 30: 8}

            def one_iteration(ham):
                # ---------- force phase (pipelined per slot) ----------
                # fpair: f2p accumulator cols [0:528], spA row at [528:1024]
                fpair = fpool.tile([128, 1024], F32, tag="fpair", name="fpair")
                spB = spbpool.tile([1, PB], F32, tag="spB", name="spB")

                for si in range(NT + LAG):
                    if si < NT:
                        if si in FL_AT:
                            emit_flight(FL_AT[si])
                        t = ORD[si]
                        js = slice(t * 128, (t + 1) * 128)
                        pair = d2pool.tile([128, R], F32, tag="d2", name="pair")
                        # d^2 = -2 r_j . r_my (dot) + sq_my + sq_j
                        if False:  # ACT-preload path: timing-unsafe, disabled
                            # Scalar preloads sq_my + sq_j into PSUM; the
                            # dots accumulate on top (PE does 2 fewer mms)
                            act(pair[:], sqmyb[:], AF.Abs,
                                bias=sqj[:, si:si + 1])
                            for clo, chi in ((0, SA), (SA, R)):
                                mm(pair[:, clo:chi], rbT[:, js],
                                   rfneg[:, clo:chi], start=False, stop=True,
                                   skip_group_check=True)
                            act(Y[:, si * R:(si + 1) * R], pair[:],
                                AF.Abs_reciprocal_sqrt)
                        else:
                            for clo, chi in ((0, SA), (SA, R)):
                                mm(pair[:, clo:chi], rbT[:, js],
                                   rfneg[:, clo:chi], start=True, stop=False)
                            for clo, chi in ((0, SA), (SA, R)):
                                mm(pair[:, clo:chi], ones_row[:],
                                   sqmyf[0:1, clo:chi], start=False, stop=True)
                            act(Y[:, si * R:(si + 1) * R], pair[:],
                                AF.Abs_reciprocal_sqrt, bias=sqj[:, si:si + 1])
                        if si % GV == GV - 1 or si == NT - 1:
                            g0 = (si // GV) * GV
                            gs = slice(g0 * R, (si + 1) * R)
                            w = (si + 1 - g0) * R
                            t2 = wpool.tile([128, GV * R], F16, tag="t2g")
                            V.tensor_tensor(Y[:, gs].bitcast(U16),
                                            Y[:, gs].bitcast(U16),
                                            maskU[:, gs], op=OP.bitwise_and)
                            V.tensor_tensor(t2[:, 0:w], Y[:, gs], Y[:, gs],
                                            op=OP.mult)
                            V.tensor_tensor(U[:, gs], t2[:, 0:w], Y[:, gs],
                                            op=OP.mult)
                    if si >= LAG:
                        k = si - LAG
                        ks = slice(k * 128, (k + 1) * 128)
                        st = (k == 0)
                        sp = (k == NT - 1)
                        u0 = k * R
                        mm(fpair[:, 0:SA], RN[:, ks], U[:, u0:u0 + SA],
                           start=st, stop=sp, skip_group_check=True)
                        mm(fpair[:, SA:R], RN[:, ks], U[:, u0 + SA:u0 + R],
                           start=st, stop=sp, skip_group_check=True)
                        mm(fpair[0:1, R:R + PA], ones_col[:], U[:, u0:u0 + PA],
                           start=st, stop=sp, skip_group_check=True)
                        mm(spB[:], ones_col[:], U[:, u0 + PA:u0 + R],
                           start=st, stop=sp, skip_group_check=True)

                # ---------- a, v, r update (full width) ----------
                # broadcast (gpsimd) runs while DVE adds the f2p term
                s_sb = wpool.tile([1, R], F32, tag="s_sb")
                V.tensor_scalar_mul(s_sb[:, 0:PA], fpair[0:1, R:R + PA], -G2)
                V.tensor_scalar_mul(s_sb[:, PA:R], spB[:], -G2)
                sbc = wpool.tile([128, R], F32, tag="sbc")
                GP.partition_broadcast(sbc[:], s_sb[:])
                V.scalar_tensor_tensor(vT[:], fpair[:, 0:R], G2,
                                       vT[:], op0=OP.mult, op1=OP.add)
                t1 = wpool.tile([128, R], F32, tag="t1")
                V.tensor_tensor(t1[:], sbc[:], rT[:], op=OP.mult)
                V.tensor_tensor(vT[:], vT[:], t1[:], op=OP.add)
                V.scalar_tensor_tensor(rT[:], vT[:], DT, rT[:],
                                       op0=OP.mult, op1=OP.add)

                # ---------- per-half update + send ----------
                rmidb = wpool.tile([128, R], F16, tag="rmidb")
                vmidb = wpool.tile([128, R], F16, tag="vmidb")
                h1b = wpool.tile([HID, R], F16, tag="h1b")
                h2b = wpool.tile([HID, R], F16, tag="h2b")
                tnh = wpool.tile([128, R], F16, tag="tnh")
                frinv = wpool.tile([1, R], F32, tag="frinv")
                sqf = wpool.tile([1, R], F32, tag="sqf")
                fr = wpool.tile([1, R], F32, tag="fr")
                fbc = wpool.tile([128, R], F32, tag="fbc")
                sqpost = wpool.tile([1, R], F32, tag="sqpost")
                r2b = wpool.tile([128, R], F16, tag="r2b")
                gaths = []

                for h, (lo, hi) in enumerate(((0, H0), (H0, H1))):
                    hw = hi - lo
                    hs = slice(lo, hi)
                    # boundary MLP on this half
                    V.tensor_copy(rmidb[:, hs], rT[:, hs])
                    V.tensor_copy(vmidb[:, hs], vT[:, hs])
                    hp1 = d2pool.tile([128, R], F32, tag="d2", name=f"hp1{h}")
                    mm(hp1[0:HID, 0:hw], bw1a[:], rmidb[:, hs],
                       start=True, stop=False)
                    mm(hp1[0:HID, 0:hw], bw1b[:], vmidb[:, hs],
                       start=False, stop=True)
                    act(h1b[:, hs], hp1[0:HID, 0:hw], AF.Tanh, bias=bb1c[:])
                    hp2 = d2pool.tile([128, R], F32, tag="d2", name=f"hp2{h}")
                    mm(hp2[0:HID, 0:hw], bw2[:], h1b[:, hs],
                       start=True, stop=True)
                    act(h2b[:, hs], hp2[0:HID, 0:hw], AF.Tanh, bias=bb2c[:])
                    hp3 = d2pool.tile([128, R], F32, tag="d2", name=f"hp3{h}")
                    mm(hp3[:, 0:hw], bw3[:], h2b[:, hs],
                       start=True, stop=True)
                    act(tnh[:, hs], hp3[:, 0:hw], AF.Tanh, bias=bb3c[:])
                    V.scalar_tensor_tensor(rT[:, hs], tnh[:, hs], 0.1,
                                           rT[:, hs], op0=OP.mult, op1=OP.add)

                    # Hamiltonian step (every 5th iteration)
                    if ham:
                        rm2b = wpool.tile([128, R], F16, tag="rm2b")
                        V.tensor_copy(rm2b[:, hs], rT[:, hs])
                        for (w1s, wTs, bcol, wcol, wcoln, src, dst, coef) in (
                            ((pw1a, pw1b), (pw1Ta, pw1Tb), pb1c, pw2c, pw2cn,
                             rm2b, rT, -0.01),
                            ((kw1a, kw1b), (kw1Ta, kw1Tb), kb1c, kw2c, kw2cn,
                             vmidb, vT, 0.01),
                        ):
                            gq = []
                            for q in range(2):
                                tq = wpool.tile([128, R], F16, tag="tq")
                                tp = d2pool.tile([128, R], F32, tag="d2",
                                                 name=f"tp{h}")
                                mm(tp[:, 0:hw], w1s[q][:], src[:, hs],
                                   start=True, stop=True)
                                act(tq[:, hs], tp[:, 0:hw], AF.Tanh,
                                    bias=bcol[:, q:q + 1])
                                t2q = wpool.tile([128, R], F16, tag="t2q")
                                V.tensor_tensor(t2q[:, hs], tq[:, hs],
                                                tq[:, hs], op=OP.mult)
                                g = wpool.tile([128, R], F16, tag=f"gq{q}",
                                               name=f"gq{q}")
                                V.tensor_scalar(g[:, hs], t2q[:, hs],
                                                wcoln[:, q:q + 1],
                                                wcol[:, q:q + 1],
                                                op0=OP.mult, op1=OP.add)
                                gq.append(g)
                            dpp = d2pool.tile([128, R], F32, tag="d2",
                                              name=f"dpp{h}")
                            for q in range(2):
                                mm(dpp[:, 0:hw], wTs[q][:], gq[q][:, hs],
                                   start=(q == 0), stop=(q == 1))
                            V.scalar_tensor_tensor(dst[:, hs], dpp[:, 0:hw],
                                                   coef, dst[:, hs],
                                                   op0=OP.mult, op1=OP.add)

                    # damping + norm clip + casts on this half
                    V.tensor_scalar_mul(vT[:, hs], vT[:, hs], DAMP)
                    act(r2b[:, hs], rT[:, hs], AF.Square)
                    sqp = d2pool.tile([128, R], F32, tag="d2", name=f"sqp{h}")
                    mm(sqp[0:1, 0:hw], ones_col[:], r2b[:, hs],
                       start=True, stop=True)
                    act(frinv[:, hs], sqp[0:1, 0:hw], AF.Abs_reciprocal_sqrt)
                    V.tensor_copy(sqf[:, hs], sqp[0:1, 0:hw])
                    V.tensor_tensor(fr[:, hs], frinv[:, hs], mnrow[:, hs],
                                    op=OP.mult)
                    V.tensor_scalar_min(fr[:, hs], fr[:, hs], 1.0)
                    GP.partition_broadcast(fbc[:, hs], fr[:, hs])
                    V.tensor_tensor(rT[:, hs], rT[:, hs], fbc[:, hs],
                                    op=OP.mult)
                    V.tensor_copy(rfinb[:, hs], rT[:, hs])
                    V.tensor_scalar_mul(rfneg[:, hs], rfinb[:, hs], -2.0)
                    V.tensor_tensor(sqpost[:, hs], sqf[:, hs], mn2row[:, hs],
                                    op=OP.min)
                    V.tensor_copy(sqmyf[:, hs], sqpost[:, hs])
                    GP.partition_broadcast(sqmyb[:, hs], sqpost[:, hs])

                    # gather this half (positions only)
                    blob = dpool.tile([128, hw], F16, tag=f"blob{h}")
                    dma(blob[:], rfinb[:, hs])
                    gath = dpool.tile([NC * 128, hw], F16, tag=f"gath{h}",
                                      addr_space="Shared")
                    nc.gpsimd.collective_compute(
                        "AllGather", OP.bypass,
                        replica_groups=[list(range(NC))],
                        ins=[blob[:].opt()], outs=[gath[:].opt()],
                    )
                    gaths.append((gath, lo, hi))

                # redistribute AFTER both collectives are in flight. Half A
                # completed during the second half's compute, so its waits
                # are cheap; half B waits only ever block the sync queue and
                # the gpsimd queue, both idle during the force phase.
                gath, lo, hi = gaths[0]
                rqa = [nc.sync, nc.sync, nc.scalar, nc.scalar,
                       nc.gpsimd, nc.gpsimd, nc.sync, nc.sync]
                for g in range(NC):
                    rqa[g].dma_start(rbT[:, g * R + lo:g * R + hi],
                                     gath[g * 128:(g + 1) * 128, :])
                gath, lo, hi = gaths[1]
                for g in range(NC):
                    nc.sync.dma_start(rbT[:, g * R + lo:g * R + hi],
                                      gath[g * 128:(g + 1) * 128, :])

            for it in range(iters):
                one_iteration(ham=(it % 5 == 0))

            dma(o_r, rT[:])

    nc.compile()
    return nc


def _prep_inputs(x, attractor_positions, attractor_masses, init_velocities,
                 pw1, pb1, pw2, pb2, kw1, kb1, kw2, kb2,
                 bw1, bb1, bw2, bb2, bw3, bb3, fw, fb):
    f16 = np.float16
    parts = np.asarray(x, np.float32).reshape(-1, D)
    ap = np.asarray(attractor_positions, np.float32)
    v0in = np.asarray(init_velocities, np.float32)
    r0 = np.concatenate([parts, np.tile(ap, (B, 1))], 0)
    v0 = np.concatenate([np.zeros_like(parts), np.tile(v0in, (B, 1))], 0)
    npad = MP - M0
    pad_pos = np.zeros((npad, D), np.float32)
    for p in range(npad):
        pad_pos[p, p % D] = 60.0 + 0.5 * p
    r0p = np.concatenate([r0, pad_pos], 0)
    v0p = np.concatenate([v0, np.zeros((npad, D), np.float32)], 0)
    mn_vec = np.concatenate([np.full((M0,), MAXNORM, np.float32),
                             np.full((npad,), 1e30, np.float32)])
    mn2_vec = np.concatenate([np.full((M0,), MAXNORM * MAXNORM, np.float32),
                              np.full((npad,), 1e30, np.float32)])

    # kill mask: diagonal + duplicated-attractor pairs + zero-mass pad rows
    kill = np.zeros((MP, MP), dtype=bool)
    kill[np.arange(MP), np.arange(MP)] = True
    kill[M0:, :] = True
    base = B * N
    for k in range(K):
        idxs = [base + b * K + k for b in range(B)]
        for a_ in idxs:
            for b_ in idxs:
                if a_ != b_:
                    kill[a_, b_] = True

    sq0 = np.sum(r0p * r0p, -1).astype(np.float32)
    rbT0 = r0p.T.astype(f16)                      # [128, MP]

    w = {
        "id128": np.eye(128, dtype=f16),
        "bw1a": np.asarray(bw1, np.float32)[0:128].astype(f16),
        "bw1b": np.asarray(bw1, np.float32)[128:256].astype(f16),
        "bw2": np.asarray(bw2, np.float32).astype(f16),
        "bw3": np.asarray(bw3, np.float32).astype(f16),
        "pw1a": np.asarray(pw1, np.float32)[:, 0:128].astype(f16),
        "pw1b": np.asarray(pw1, np.float32)[:, 128:256].astype(f16),
        "pw1Ta": np.ascontiguousarray(np.asarray(pw1, np.float32).T[0:128]).astype(f16),
        "pw1Tb": np.ascontiguousarray(np.asarray(pw1, np.float32).T[128:256]).astype(f16),
        "kw1a": np.asarray(kw1, np.float32)[:, 0:128].astype(f16),
        "kw1b": np.asarray(kw1, np.float32)[:, 128:256].astype(f16),
        "kw1Ta": np.ascontiguousarray(np.asarray(kw1, np.float32).T[0:128]).astype(f16),
        "kw1Tb": np.ascontiguousarray(np.asarray(kw1, np.float32).T[128:256]).astype(f16),
        "bb1c": np.asarray(bb1, np.float32).reshape(HID, 1),
        "bb2c": np.asarray(bb2, np.float32).reshape(HID, 1),
        "bb3c": np.asarray(bb3, np.float32).reshape(128, 1),
        "pb1c": np.asarray(pb1, np.float32).reshape(2, 128).T.copy(),
        "kb1c": np.asarray(kb1, np.float32).reshape(2, 128).T.copy(),
        "pw2c": np.asarray(pw2, np.float32).reshape(2, 128).T.copy(),
        "pw2cn": (-np.asarray(pw2, np.float32)).reshape(2, 128).T.copy(),
        "kw2c": np.asarray(kw2, np.float32).reshape(2, 128).T.copy(),
        "kw2cn": (-np.asarray(kw2, np.float32)).reshape(2, 128).T.copy(),
    }

    in_maps = []
    for c in range(NC):
        rows = slice(c * R, (c + 1) * R)
        m = dict(w)
        m["rT0"] = np.ascontiguousarray(r0p[rows].T)
        m["vT0"] = np.ascontiguousarray(v0p[rows].T)
        m["rbT0"] = rbT0
        m["rfin0"] = np.ascontiguousarray(rbT0[:, rows])
        m["sqrow0"] = sq0[rows].reshape(1, R).astype(np.float32)
        kl = kill[:, rows]  # [MP, R] -> slot s holds tile ORD[s]: [128, R]
        mask = np.where(kl, np.uint16(0), np.uint16(0xFFFF))
        maskU = np.concatenate(
            [mask[t * 128:(t + 1) * 128, :] for t in ORD], axis=1)
        m["maskU0"] = np.ascontiguousarray(maskU)
        m["mnrow"] = mn_vec[rows].reshape(1, R)
        m["mn2row"] = mn2_vec[rows].reshape(1, R)
        in_maps.append(m)
    return in_maps


def _epilogue(r_fin, attractor_masses, fw, fb):
    fp = r_fin[:B * N].reshape(B, N, D)
    fa = r_fin[B * N:M0].reshape(B, K, D)
    am = np.asarray(attractor_masses, np.float32)
    pm = np.ones((B, N), np.float32)
    am2 = np.broadcast_to(am[None, :], (B, K))
    allm = np.concatenate([pm, am2], 1)
    allp = np.concatenate([fp, fa], 1)
    tot = allm.sum(1, keepdims=True)
    com = (allm[..., None] * allp).sum(1) / tot
    centered = allp - com[:, None, :]
    var = (centered ** 2).mean(1)
    skew = (centered ** 3).mean(1)
    p0 = allp[0]
    sq0 = np.sum(p0 * p0, -1)
    dmat = np.sqrt(np.maximum(sq0[:, None] + sq0[None, :] - 2.0 * p0 @ p0.T, 0.0))
    iu, ju = np.triu_indices(p0.shape[0], 1)
    dd = dmat[iu, ju]
    st = np.stack([dd.mean(), dd.std(ddof=1), dd.min(), dd.max()])
    stb = np.broadcast_to(st[None, :], (B, 4)).astype(np.float32)
    feat = np.concatenate([com, var, skew, stb], -1)
    return (feat @ np.asarray(fw, np.float32) + np.asarray(fb, np.float32)).astype(np.float32)


def _wire_ntff_hook():
    import types
    try:
        import antenv.axon_hooks  # noqa: F401
        return True
    except ImportError:
        pass
    try:
        import antenv
        from trn_agent_boot.trn_boot import _ntff_profile_via_ctypes
        mod = types.ModuleType("antenv.axon_hooks")
        _h = [None]
        mod.set_axon_ntff_profile_hook = lambda h: _h.__setitem__(0, h)
        mod.get_axon_ntff_profile_hook = lambda: _h[0]
        sys.modules["antenv.axon_hooks"] = mod
        antenv.axon_hooks = mod
        mod.set_axon_ntff_profile_hook(
            _ntff_profile_via_ctypes("/opt/axon/libaxon_pjrt.so"))
        return True
    except Exception as e:
        print(f"ntff hook wiring failed ({e}); running without trace")
        return False


def kernel(**inputs):
    from concourse.bass_utils import run_bass_kernel_spmd

    iters = int(os.environ.get("KERNEL_ITERS", ITERS))
    trace = bool(int(os.environ.get("KERNEL_TRACE", "0")))
    if trace:
        trace = _wire_ntff_hook()
    if iters not in _PROGRAM_CACHE:
        _PROGRAM_CACHE[iters] = _build_program(iters)
    nc = _PROGRAM_CACHE[iters]
    in_maps = _prep_inputs(**inputs)
    res = run_bass_kernel_spmd(nc, in_maps, core_ids=list(range(NC)), trace=trace)
    if trace and res.exec_time_ns is not None:
        print(f"HW exec time: {res.exec_time_ns} ns")
    rT_full = np.zeros((128, MP), np.float32)
    for c in range(NC):
        rT_full[:, c * R:(c + 1) * R] = res.results[c]["r_out"]
    r_fin = np.ascontiguousarray(rT_full.T)[:M0]
    out = _epilogue(r_fin, inputs["attractor_masses"], inputs["fw"], inputs["fb"])
    kernel.last_exec_time_ns = res.exec_time_ns if trace else None
    return out


if __name__ == "__main__":
    rng = np.random.default_rng(0)
    demo = {
        "x": rng.standard_normal((B, N, D)).astype(np.float32),
        "attractor_positions": rng.standard_normal((K, D)).astype(np.float32),
        "attractor_masses": np.full((K,), 0.1, np.float32),
        "init_velocities": np.zeros((K, D), np.float32),
        "pw1": rng.standard_normal((D, 2 * D)).astype(np.float32) / 16,
        "pb1": np.zeros((2 * D,), np.float32),
        "pw2": rng.standard_normal((2 * D, 1)).astype(np.float32) / 16,
        "pb2": np.zeros((1,), np.float32),
        "kw1": rng.standard_normal((D, 2 * D)).astype(np.float32) / 16,
        "kb1": np.zeros((2 * D,), np.float32),
        "kw2": rng.standard_normal((2 * D, 1)).astype(np.float32) / 16,
        "kb2": np.zeros((1,), np.float32),
        "bw1": rng.standard_normal((2 * D, HID)).astype(np.float32) / 16,
        "bb1": np.zeros((HID,), np.float32),
        "bw2": rng.standard_normal((HID, HID)).astype(np.float32) / 8,
        "bb2": np.zeros((HID,), np.float32),
        "bw3": rng.standard_normal((HID, D)).astype(np.float32) / 8,
        "bb3": np.zeros((D,), np.float32),
        "fw": rng.standard_normal((3 * D + 4, D)).astype(np.float32) / 20,
        "fb": np.zeros((D,), np.float32),
    }
    out = kernel(**demo)
    print("out", out.shape, np.abs(out).max())


# revision 52
# speedup vs baseline: 1.2185x; 1.2185x over previous
"""Trainium2 Bass kernel for nn_AttractorCore (all-pairs N-body + MLP dynamics).

Self-contained: hardcodes shapes/sharding. Accepts FULL inputs, returns FULL output.
Distribution: data-parallel over bodies. M=4136 bodies padded to 4224 = 33*128;
each of 8 cores owns 528 rows. Per iteration each core computes its [528 x 4224]
tile of the interaction matrix (f16 matmuls; d^2 assembled by preloading
sq_j + sq_my into PSUM on GpSimd and accumulating -2*r_j.r_my on top, so the
old [2,128]-stationary sq matmuls are gone), takes d^-1 via one wide ACT per
j-tile, cubes/masks on DVE, and aggregates sum(w*r_j) plus sum(w) into spare
PSUM regions. PSUM is organized as [128,528] two-bank pairs with asymmetric
512/16 matmul splits so every ACT/preload is a single wide op. The j-major
position layout is rebuilt with PE transposes; per-flight DVE square+reduce
produces the j-major squared norms locally, so the AllGather ships positions
only. Final projection/stats epilogue on host.
"""
import os
import sys
import numpy as np
import ml_dtypes

sys.path.insert(0, "/opt/trn_rl_repo")

# ---- problem constants (hardcoded) ----
B, N, D, K = 4, 1024, 128, 10
G, DAMP, DT, ITERS, EPS, MAXNORM, HID = 1e-3, 0.95, 0.1, 50, 1e-6, 10.0, 64
M0 = B * N + B * K          # 4136 real bodies
NT = 33                     # j tiles of 128
MP = NT * 128               # 4224 padded bodies
NC = 8                      # cores
R = MP // NC                # 528 rows per core
G2 = G * DT * 0.1           # gravity * dt * uniform body mass (folded)
SA, SB = 512, 16            # asymmetric my split (PSUM bank containment)
PA, PB = 496, 32            # sp row split (bank tails)
H0, H1 = 256, R             # update-phase halves [0:256) / [256:528)
EPS2 = 1e-3                 # d^2 floor: keeps self-pair 1/d^3 finite (their
                            # force contribution cancels in f2p - sp*r)
# force-phase j-tile processing order: tiles fully covered by every core's
# first-half columns [0:256) come first so they can start while the
# second-half AllGather is still in flight. Slot s stores tile ORD[s].
ORD = [0, 1, 5, 9, 13, 17, 21, 25, 29,
       2, 3, 4, 6, 7, 8, 10, 11, 12, 14, 15, 16, 18, 19, 20,
       22, 23, 24, 26, 27, 28, 30, 31, 32]

_PROGRAM_CACHE = {}


def _build_program(iters):
    import concourse.bacc as bacc
    import concourse.mybir as mybir
    import concourse.tile as tile

    dt = mybir.dt
    F32, F16, U16 = dt.float32, dt.float16, dt.uint16
    AF = mybir.ActivationFunctionType
    OP = mybir.AluOpType

    nc = bacc.Bacc("TRN2", target_bir_lowering=False, debug=False, num_devices=NC)

    def din(name, shape, dtype):
        return nc.dram_tensor(name, shape, dtype, kind="ExternalInput").ap()

    i_rT0 = din("rT0", [128, R], F32)
    i_vT0 = din("vT0", [128, R], F32)
    i_rbT0 = din("rbT0", [128, MP], F16)
    i_rfin0 = din("rfin0", [128, R], F16)
    i_sqrow0 = din("sqrow0", [1, R], F32)     # |r0|^2 per my column
    i_maskU0 = din("maskU0", [128, NT * R], U16)
    i_mnrow = din("mnrow", [1, R], F32)       # max-norm per my column (1e30 pads)
    i_mn2row = din("mn2row", [1, R], F32)     # cap for sq
    i_id128 = din("id128", [128, 128], F16)
    i_bw1a = din("bw1a", [128, HID], F16)
    i_bw1b = din("bw1b", [128, HID], F16)
    i_bw2 = din("bw2", [HID, HID], F16)
    i_bw3 = din("bw3", [HID, 128], F16)
    i_pw1a = din("pw1a", [128, 128], F16)
    i_pw1b = din("pw1b", [128, 128], F16)
    i_pw1Ta = din("pw1Ta", [128, 128], F16)
    i_pw1Tb = din("pw1Tb", [128, 128], F16)
    i_kw1a = din("kw1a", [128, 128], F16)
    i_kw1b = din("kw1b", [128, 128], F16)
    i_kw1Ta = din("kw1Ta", [128, 128], F16)
    i_kw1Tb = din("kw1Tb", [128, 128], F16)
    i_bb1c = din("bb1c", [HID, 1], F32)
    i_bb2c = din("bb2c", [HID, 1], F32)
    i_bb3c = din("bb3c", [128, 1], F32)
    i_pb1c = din("pb1c", [128, 2], F32)       # col q
    i_kb1c = din("kb1c", [128, 2], F32)
    i_pw2c = din("pw2c", [128, 2], F32)
    i_pw2cn = din("pw2cn", [128, 2], F32)
    i_kw2c = din("kw2c", [128, 2], F32)
    i_kw2cn = din("kw2cn", [128, 2], F32)

    o_r = nc.dram_tensor("r_out", [128, R], F32, kind="ExternalOutput").ap()

    with tile.TileContext(nc) as tc:
        with (
            tc.tile_pool(name="cpool", bufs=1) as cpool,
            tc.tile_pool(name="wpool", bufs=1) as wpool,
            tc.tile_pool(name="d2pool", bufs=2, space="PSUM") as d2pool,
            tc.tile_pool(name="fpool", bufs=1, space="PSUM") as fpool,
            tc.tile_pool(name="spbpool", bufs=1, space="PSUM") as spbpool,
            tc.tile_pool(name="rnpool", bufs=1, space="PSUM") as rnpool,
            tc.tile_pool(name="dpool", bufs=3, space="DRAM") as dpool,
        ):
            # ---- persistent tiles ----
            rT = cpool.tile([128, R], F32, tag="rT")
            vT = cpool.tile([128, R], F32, tag="vT")
            rbT = cpool.tile([128, MP], F16, tag="rbT")
            RN = cpool.tile([128, MP], F16, tag="RN")       # j-major positions
            Y = cpool.tile([128, NT * R], F16, tag="Y")     # 1/d
            U = cpool.tile([128, NT * R], F16, tag="U")     # 1/d^3
            maskU = cpool.tile([128, NT * R], U16, tag="maskU")
            rfinb = cpool.tile([128, R], F16, tag="rfinb")
            rfneg = cpool.tile([128, R], F16, tag="rfneg")  # -2 * rfinb
            sqmyf = cpool.tile([1, R], F16, tag="sqmyf")    # |r_my|^2 row
            sqmyb = cpool.tile([128, R], F32, tag="sqmyb")  # bcast |r_my|^2
            sqrow = cpool.tile([1, R], F32, tag="sqrow")
            sqj = cpool.tile([128, NT], F32, tag="sqj")     # j-major |r_j|^2
            ones_row = cpool.tile([1, 128], F16, tag="ones_row")
            mnrow = cpool.tile([1, R], F32, tag="mnrow")
            mn2row = cpool.tile([1, R], F32, tag="mn2row")
            id128 = cpool.tile([128, 128], F16, tag="id128")
            ones_col = cpool.tile([128, 1], F16, tag="ones_col")
            bw1a = cpool.tile([128, HID], F16, tag="bw1a")
            bw1b = cpool.tile([128, HID], F16, tag="bw1b")
            bw2 = cpool.tile([HID, HID], F16, tag="bw2")
            bw3 = cpool.tile([HID, 128], F16, tag="bw3")
            pw1a = cpool.tile([128, 128], F16, tag="pw1a")
            pw1b = cpool.tile([128, 128], F16, tag="pw1b")
            pw1Ta = cpool.tile([128, 128], F16, tag="pw1Ta")
            pw1Tb = cpool.tile([128, 128], F16, tag="pw1Tb")
            kw1a = cpool.tile([128, 128], F16, tag="kw1a")
            kw1b = cpool.tile([128, 128], F16, tag="kw1b")
            kw1Ta = cpool.tile([128, 128], F16, tag="kw1Ta")
            kw1Tb = cpool.tile([128, 128], F16, tag="kw1Tb")
            bb1c = cpool.tile([HID, 1], F32, tag="bb1c")
            bb2c = cpool.tile([HID, 1], F32, tag="bb2c")
            bb3c = cpool.tile([128, 1], F32, tag="bb3c")
            pb1c = cpool.tile([128, 2], F32, tag="pb1c")
            kb1c = cpool.tile([128, 2], F32, tag="kb1c")
            pw2c = cpool.tile([128, 2], F32, tag="pw2c")
            pw2cn = cpool.tile([128, 2], F32, tag="pw2cn")
            kw2c = cpool.tile([128, 2], F32, tag="kw2c")
            kw2cn = cpool.tile([128, 2], F32, tag="kw2cn")

            dma = nc.sync.dma_start
            mm = nc.tensor.matmul
            act = nc.scalar.activation
            V = nc.vector
            GP = nc.gpsimd

            # ---- one-time loads ----
            for dst, src in [
                (rT, i_rT0), (vT, i_vT0), (rbT, i_rbT0), (rfinb, i_rfin0),
                (sqrow, i_sqrow0), (maskU, i_maskU0),
                (mnrow, i_mnrow), (mn2row, i_mn2row), (id128, i_id128),
                (bw1a, i_bw1a), (bw1b, i_bw1b), (bw2, i_bw2), (bw3, i_bw3),
                (pw1a, i_pw1a), (pw1b, i_pw1b), (pw1Ta, i_pw1Ta), (pw1Tb, i_pw1Tb),
                (kw1a, i_kw1a), (kw1b, i_kw1b), (kw1Ta, i_kw1Ta), (kw1Tb, i_kw1Tb),
                (bb1c, i_bb1c), (bb2c, i_bb2c), (bb3c, i_bb3c),
                (pb1c, i_pb1c), (kb1c, i_kb1c),
                (pw2c, i_pw2c), (pw2cn, i_pw2cn), (kw2c, i_kw2c), (kw2cn, i_kw2cn),
            ]:
                dma(dst[:], src)
            nc.gpsimd.memset(ones_col[:], 1.0)
            nc.gpsimd.memset(ones_row[:], 1.0)
            # initial derived state
            V.tensor_scalar_mul(rfneg[:], rfinb[:], -2.0)
            V.tensor_copy(sqmyf[:], sqrow[:])
            GP.partition_broadcast(sqmyb[:], sqrow[:])

            LAG = 6     # j-tiles between d2/act emission and f2p consumption
            GV = 4      # j-tiles per DVE mask/square/cube group

            def emit_flight(f):
                # PE-transpose up to 8 j-tiles of rbT into PSUM (slot order),
                # one wide DVE copy into the slot-major RN layout, then
                # square+reduce for the slot-major squared norms.
                rnp = rnpool.tile([128, 1024], F16, tag="rnp")
                lo = 8 * f
                hi = min(lo + 8, NT)
                for s in range(lo, hi):
                    t = ORD[s]
                    nc.tensor.transpose(
                        rnp[:, (s - lo) * 128:(s - lo + 1) * 128],
                        rbT[:, t * 128:(t + 1) * 128], id128[:])
                w = (hi - lo) * 128
                # Scalar does the PSUM->SBUF copy: keeps it off the DVE
                # backlog so the next flight's transposes aren't WAR-stalled
                act(RN[:, lo * 128:lo * 128 + w], rnp[:, 0:w], AF.Copy)
                rnsq = wpool.tile([128, 1024], F16, tag="rnsq")
                V.tensor_tensor(rnsq[:, 0:w], RN[:, lo * 128:lo * 128 + w],
                                RN[:, lo * 128:lo * 128 + w], op=OP.mult)
                V.tensor_reduce(
                    sqj[:, lo:hi],
                    rnsq[:, 0:w].rearrange("p (t c) -> p t c", c=128),
                    axis=mybir.AxisListType.X, op=OP.add)

            FL_AT = {0: 0, 8: 1, 13: 2, 19: 3, 26: 4}

            def one_iteration(ham):
                # ---------- force phase (pipelined per slot) ----------
                # fpair: f2p accumulator cols [0:528], spA row at [528:1024]
                fpair = fpool.tile([128, 1024], F32, tag="fpair", name="fpair")
                spB = spbpool.tile([1, PB], F32, tag="spB", name="spB")

                for si in range(NT + LAG):
                    if si < NT:
                        if si in FL_AT and si > 0:
                            emit_flight(FL_AT[si])
                        t = ORD[si]
                        js = slice(t * 128, (t + 1) * 128)
                        pair = d2pool.tile([128, R], F32, tag="d2", name="pair")
                        # d^2 = -2 r_j . r_my (dot) + sq_my + sq_j
                        if False:  # ACT-preload path: timing-unsafe, disabled
                            # Scalar preloads sq_my + sq_j into PSUM; the
                            # dots accumulate on top (PE does 2 fewer mms)
                            act(pair[:], sqmyb[:], AF.Abs,
                                bias=sqj[:, si:si + 1])
                            for clo, chi in ((0, SA), (SA, R)):
                                mm(pair[:, clo:chi], rbT[:, js],
                                   rfneg[:, clo:chi], start=False, stop=True,
                                   skip_group_check=True)
                            act(Y[:, si * R:(si + 1) * R], pair[:],
                                AF.Abs_reciprocal_sqrt)
                        else:
                            for clo, chi in ((0, SA), (SA, R)):
                                mm(pair[:, clo:chi], rbT[:, js],
                                   rfneg[:, clo:chi], start=True, stop=False)
                            for clo, chi in ((0, SA), (SA, R)):
                                mm(pair[:, clo:chi], ones_row[:],
                                   sqmyf[0:1, clo:chi], start=False, stop=True)
                            if si == 0:
                                # flight 0 after slot 0's mms: the first dot
                                # waits only its own tile's redistribution
                                emit_flight(0)
                            act(Y[:, si * R:(si + 1) * R], pair[:],
                                AF.Abs_reciprocal_sqrt, bias=sqj[:, si:si + 1])
                        if si % GV == GV - 1 or si == NT - 1:
                            g0 = (si // GV) * GV
                            gs = slice(g0 * R, (si + 1) * R)
                            w = (si + 1 - g0) * R
                            t2 = wpool.tile([128, GV * R], F16, tag="t2g")
                            V.tensor_tensor(Y[:, gs].bitcast(U16),
                                            Y[:, gs].bitcast(U16),
                                            maskU[:, gs], op=OP.bitwise_and)
                            V.tensor_tensor(t2[:, 0:w], Y[:, gs], Y[:, gs],
                                            op=OP.mult)
                            V.tensor_tensor(U[:, gs], t2[:, 0:w], Y[:, gs],
                                            op=OP.mult)
                    if si >= LAG:
                        k = si - LAG
                        ks = slice(k * 128, (k + 1) * 128)
                        st = (k == 0)
                        sp = (k == NT - 1)
                        u0 = k * R
                        mm(fpair[:, 0:SA], RN[:, ks], U[:, u0:u0 + SA],
                           start=st, stop=sp, skip_group_check=True)
                        mm(fpair[:, SA:R], RN[:, ks], U[:, u0 + SA:u0 + R],
                           start=st, stop=sp, skip_group_check=True)
                        mm(fpair[0:1, R:R + PA], ones_col[:], U[:, u0:u0 + PA],
                           start=st, stop=sp, skip_group_check=True)
                        mm(spB[:], ones_col[:], U[:, u0 + PA:u0 + R],
                           start=st, stop=sp, skip_group_check=True)

                # ---------- a, v, r update (full width) ----------
                # broadcast (gpsimd) runs while DVE adds the f2p term
                s_sb = wpool.tile([1, R], F32, tag="s_sb")
                V.tensor_scalar_mul(s_sb[:, 0:PA], fpair[0:1, R:R + PA], -G2)
                V.tensor_scalar_mul(s_sb[:, PA:R], spB[:], -G2)
                sbc = wpool.tile([128, R], F32, tag="sbc")
                GP.partition_broadcast(sbc[:], s_sb[:])
                V.scalar_tensor_tensor(vT[:], fpair[:, 0:R], G2,
                                       vT[:], op0=OP.mult, op1=OP.add)
                t1 = wpool.tile([128, R], F32, tag="t1")
                V.tensor_tensor(t1[:], sbc[:], rT[:], op=OP.mult)
                V.tensor_tensor(vT[:], vT[:], t1[:], op=OP.add)
                V.scalar_tensor_tensor(rT[:], vT[:], DT, rT[:],
                                       op0=OP.mult, op1=OP.add)

                # ---------- per-half update + send ----------
                rmidb = wpool.tile([128, R], F16, tag="rmidb")
                vmidb = wpool.tile([128, R], F16, tag="vmidb")
                h1b = wpool.tile([HID, R], F16, tag="h1b")
                h2b = wpool.tile([HID, R], F16, tag="h2b")
                tnh = wpool.tile([128, R], F16, tag="tnh")
                frinv = wpool.tile([1, R], F32, tag="frinv")
                sqf = wpool.tile([1, R], F32, tag="sqf")
                fr = wpool.tile([1, R], F32, tag="fr")
                fbc = wpool.tile([128, R], F32, tag="fbc")
                sqpost = wpool.tile([1, R], F32, tag="sqpost")
                r2b = wpool.tile([128, R], F16, tag="r2b")
                gaths = []

                for h, (lo, hi) in enumerate(((0, H0), (H0, H1))):
                    hw = hi - lo
                    hs = slice(lo, hi)
                    # boundary MLP on this half
                    V.tensor_copy(rmidb[:, hs], rT[:, hs])
                    V.tensor_copy(vmidb[:, hs], vT[:, hs])
                    hp1 = d2pool.tile([128, R], F32, tag="d2", name=f"hp1{h}")
                    mm(hp1[0:HID, 0:hw], bw1a[:], rmidb[:, hs],
                       start=True, stop=False)
                    mm(hp1[0:HID, 0:hw], bw1b[:], vmidb[:, hs],
                       start=False, stop=True)
                    act(h1b[:, hs], hp1[0:HID, 0:hw], AF.Tanh, bias=bb1c[:])
                    hp2 = d2pool.tile([128, R], F32, tag="d2", name=f"hp2{h}")
                    mm(hp2[0:HID, 0:hw], bw2[:], h1b[:, hs],
                       start=True, stop=True)
                    act(h2b[:, hs], hp2[0:HID, 0:hw], AF.Tanh, bias=bb2c[:])
                    hp3 = d2pool.tile([128, R], F32, tag="d2", name=f"hp3{h}")
                    mm(hp3[:, 0:hw], bw3[:], h2b[:, hs],
                       start=True, stop=True)
                    act(tnh[:, hs], hp3[:, 0:hw], AF.Tanh, bias=bb3c[:])
                    V.scalar_tensor_tensor(rT[:, hs], tnh[:, hs], 0.1,
                                           rT[:, hs], op0=OP.mult, op1=OP.add)

                    # Hamiltonian step (every 5th iteration)
                    if ham:
                        rm2b = wpool.tile([128, R], F16, tag="rm2b")
                        V.tensor_copy(rm2b[:, hs], rT[:, hs])
                        for (w1s, wTs, bcol, wcol, wcoln, src, dst, coef) in (
                            ((pw1a, pw1b), (pw1Ta, pw1Tb), pb1c, pw2c, pw2cn,
                             rm2b, rT, -0.01),
                            ((kw1a, kw1b), (kw1Ta, kw1Tb), kb1c, kw2c, kw2cn,
                             vmidb, vT, 0.01),
                        ):
                            gq = []
                            for q in range(2):
                                tq = wpool.tile([128, R], F16, tag="tq")
                                tp = d2pool.tile([128, R], F32, tag="d2",
                                                 name=f"tp{h}")
                                mm(tp[:, 0:hw], w1s[q][:], src[:, hs],
                                   start=True, stop=True)
                                act(tq[:, hs], tp[:, 0:hw], AF.Tanh,
                                    bias=bcol[:, q:q + 1])
                                t2q = wpool.tile([128, R], F16, tag="t2q")
                                V.tensor_tensor(t2q[:, hs], tq[:, hs],
                                                tq[:, hs], op=OP.mult)
                                g = wpool.tile([128, R], F16, tag=f"gq{q}",
                                               name=f"gq{q}")
                                V.tensor_scalar(g[:, hs], t2q[:, hs],
                                                wcoln[:, q:q + 1],
                                                wcol[:, q:q + 1],
                                                op0=OP.mult, op1=OP.add)
                                gq.append(g)
                            dpp = d2pool.tile([128, R], F32, tag="d2",
                                              name=f"dpp{h}")
                            for q in range(2):
                                mm(dpp[:, 0:hw], wTs[q][:], gq[q][:, hs],
                                   start=(q == 0), stop=(q == 1))
                            V.scalar_tensor_tensor(dst[:, hs], dpp[:, 0:hw],
                                                   coef, dst[:, hs],
                                                   op0=OP.mult, op1=OP.add)

                    # damping + norm clip + casts on this half
                    V.tensor_scalar_mul(vT[:, hs], vT[:, hs], DAMP)
                    act(r2b[:, hs], rT[:, hs], AF.Square)
                    sqp = d2pool.tile([128, R], F32, tag="d2", name=f"sqp{h}")
                    mm(sqp[0:1, 0:hw], ones_col[:], r2b[:, hs],
                       start=True, stop=True)
                    act(frinv[:, hs], sqp[0:1, 0:hw], AF.Abs_reciprocal_sqrt)
                    V.tensor_copy(sqf[:, hs], sqp[0:1, 0:hw])
                    V.tensor_tensor(fr[:, hs], frinv[:, hs], mnrow[:, hs],
                                    op=OP.mult)
                    V.tensor_scalar_min(fr[:, hs], fr[:, hs], 1.0)
                    GP.partition_broadcast(fbc[:, hs], fr[:, hs])
                    V.tensor_tensor(rT[:, hs], rT[:, hs], fbc[:, hs],
                                    op=OP.mult)
                    V.tensor_copy(rfinb[:, hs], rT[:, hs])
                    V.tensor_scalar_mul(rfneg[:, hs], rfinb[:, hs], -2.0)
                    V.tensor_tensor(sqpost[:, hs], sqf[:, hs], mn2row[:, hs],
                                    op=OP.min)
                    V.tensor_copy(sqmyf[:, hs], sqpost[:, hs])
                    GP.partition_broadcast(sqmyb[:, hs], sqpost[:, hs])

                    # gather this half (positions only)
                    blob = dpool.tile([128, hw], F16, tag=f"blob{h}")
                    dma(blob[:], rfinb[:, hs])
                    gath = dpool.tile([NC * 128, hw], F16, tag=f"gath{h}",
                                      addr_space="Shared")
                    nc.gpsimd.collective_compute(
                        "AllGather", OP.bypass,
                        replica_groups=[list(range(NC))],
                        ins=[blob[:].opt()], outs=[gath[:].opt()],
                    )
                    gaths.append((gath, lo, hi))

                # redistribute AFTER both collectives are in flight. Half A
                # completed during the second half's compute, so its waits
                # are cheap; half B waits only ever block the sync queue and
                # the gpsimd queue, both idle during the force phase.
                gath, lo, hi = gaths[0]
                rqa = [nc.sync, nc.sync, nc.scalar, nc.scalar,
                       nc.gpsimd, nc.gpsimd, nc.sync, nc.sync]
                for g in range(NC):
                    rqa[g].dma_start(rbT[:, g * R + lo:g * R + hi],
                                     gath[g * 128:(g + 1) * 128, :])
                gath, lo, hi = gaths[1]
                for g in range(NC):
                    nc.sync.dma_start(rbT[:, g * R + lo:g * R + hi],
                                      gath[g * 128:(g + 1) * 128, :])

            for it in range(iters):
                one_iteration(ham=(it % 5 == 0))

            dma(o_r, rT[:])

    nc.compile()
    return nc


def _prep_inputs(x, attractor_positions, attractor_masses, init_velocities,
                 pw1, pb1, pw2, pb2, kw1, kb1, kw2, kb2,
                 bw1, bb1, bw2, bb2, bw3, bb3, fw, fb):
    f16 = np.float16
    parts = np.asarray(x, np.float32).reshape(-1, D)
    ap = np.asarray(attractor_positions, np.float32)
    v0in = np.asarray(init_velocities, np.float32)
    r0 = np.concatenate([parts, np.tile(ap, (B, 1))], 0)
    v0 = np.concatenate([np.zeros_like(parts), np.tile(v0in, (B, 1))], 0)
    npad = MP - M0
    pad_pos = np.zeros((npad, D), np.float32)
    for p in range(npad):
        pad_pos[p, p % D] = 60.0 + 0.5 * p
    r0p = np.concatenate([r0, pad_pos], 0)
    v0p = np.concatenate([v0, np.zeros((npad, D), np.float32)], 0)
    mn_vec = np.concatenate([np.full((M0,), MAXNORM, np.float32),
                             np.full((npad,), 1e30, np.float32)])
    mn2_vec = np.concatenate([np.full((M0,), MAXNORM * MAXNORM, np.float32),
                              np.full((npad,), 1e30, np.float32)])

    # kill mask: diagonal + duplicated-attractor pairs + zero-mass pad rows
    kill = np.zeros((MP, MP), dtype=bool)
    kill[np.arange(MP), np.arange(MP)] = True
    kill[M0:, :] = True
    base = B * N
    for k in range(K):
        idxs = [base + b * K + k for b in range(B)]
        for a_ in idxs:
            for b_ in idxs:
                if a_ != b_:
                    kill[a_, b_] = True

    sq0 = np.sum(r0p * r0p, -1).astype(np.float32)
    rbT0 = r0p.T.astype(f16)                      # [128, MP]

    w = {
        "id128": np.eye(128, dtype=f16),
        "bw1a": np.asarray(bw1, np.float32)[0:128].astype(f16),
        "bw1b": np.asarray(bw1, np.float32)[128:256].astype(f16),
        "bw2": np.asarray(bw2, np.float32).astype(f16),
        "bw3": np.asarray(bw3, np.float32).astype(f16),
        "pw1a": np.asarray(pw1, np.float32)[:, 0:128].astype(f16),
        "pw1b": np.asarray(pw1, np.float32)[:, 128:256].astype(f16),
        "pw1Ta": np.ascontiguousarray(np.asarray(pw1, np.float32).T[0:128]).astype(f16),
        "pw1Tb": np.ascontiguousarray(np.asarray(pw1, np.float32).T[128:256]).astype(f16),
        "kw1a": np.asarray(kw1, np.float32)[:, 0:128].astype(f16),
        "kw1b": np.asarray(kw1, np.float32)[:, 128:256].astype(f16),
        "kw1Ta": np.ascontiguousarray(np.asarray(kw1, np.float32).T[0:128]).astype(f16),
        "kw1Tb": np.ascontiguousarray(np.asarray(kw1, np.float32).T[128:256]).astype(f16),
        "bb1c": np.asarray(bb1, np.float32).reshape(HID, 1),
        "bb2c": np.asarray(bb2, np.float32).reshape(HID, 1),
        "bb3c": np.asarray(bb3, np.float32).reshape(128, 1),
        "pb1c": np.asarray(pb1, np.float32).reshape(2, 128).T.copy(),
        "kb1c": np.asarray(kb1, np.float32).reshape(2, 128).T.copy(),
        "pw2c": np.asarray(pw2, np.float32).reshape(2, 128).T.copy(),
        "pw2cn": (-np.asarray(pw2, np.float32)).reshape(2, 128).T.copy(),
        "kw2c": np.asarray(kw2, np.float32).reshape(2, 128).T.copy(),
        "kw2cn": (-np.asarray(kw2, np.float32)).reshape(2, 128).T.copy(),
    }

    in_maps = []
    for c in range(NC):
        rows = slice(c * R, (c + 1) * R)
        m = dict(w)
        m["rT0"] = np.ascontiguousarray(r0p[rows].T)
        m["vT0"] = np.ascontiguousarray(v0p[rows].T)
        m["rbT0"] = rbT0
        m["rfin0"] = np.ascontiguousarray(rbT0[:, rows])
        m["sqrow0"] = sq0[rows].reshape(1, R).astype(np.float32)
        kl = kill[:, rows]  # [MP, R] -> slot s holds tile ORD[s]: [128, R]
        mask = np.where(kl, np.uint16(0), np.uint16(0xFFFF))
        maskU = np.concatenate(
            [mask[t * 128:(t + 1) * 128, :] for t in ORD], axis=1)
        m["maskU0"] = np.ascontiguousarray(maskU)
        m["mnrow"] = mn_vec[rows].reshape(1, R)
        m["mn2row"] = mn2_vec[rows].reshape(1, R)
        in_maps.append(m)
    return in_maps


def _epilogue(r_fin, attractor_masses, fw, fb):
    fp = r_fin[:B * N].reshape(B, N, D)
    fa = r_fin[B * N:M0].reshape(B, K, D)
    am = np.asarray(attractor_masses, np.float32)
    pm = np.ones((B, N), np.float32)
    am2 = np.broadcast_to(am[None, :], (B, K))
    allm = np.concatenate([pm, am2], 1)
    allp = np.concatenate([fp, fa], 1)
    tot = allm.sum(1, keepdims=True)
    com = (allm[..., None] * allp).sum(1) / tot
    centered = allp - com[:, None, :]
    var = (centered ** 2).mean(1)
    skew = (centered ** 3).mean(1)
    p0 = allp[0]
    sq0 = np.sum(p0 * p0, -1)
    dmat = np.sqrt(np.maximum(sq0[:, None] + sq0[None, :] - 2.0 * p0 @ p0.T, 0.0))
    iu, ju = np.triu_indices(p0.shape[0], 1)
    dd = dmat[iu, ju]
    st = np.stack([dd.mean(), dd.std(ddof=1), dd.min(), dd.max()])
    stb = np.broadcast_to(st[None, :], (B, 4)).astype(np.float32)
    feat = np.concatenate([com, var, skew, stb], -1)
    return (feat @ np.asarray(fw, np.float32) + np.asarray(fb, np.float32)).astype(np.float32)


def _wire_ntff_hook():
    import types
    try:
        import antenv.axon_hooks  # noqa: F401
        return True
    except ImportError:
        pass
    try:
        import antenv
        from trn_agent_boot.trn_boot import _ntff_profile_via_ctypes
        mod = types.ModuleType("antenv.axon_hooks")
        _h = [None]
        mod.set_axon_ntff_profile_hook = lambda h: _h.__setitem__(0, h)
        mod.get_axon_ntff_profile_hook = lambda: _h[0]
        sys.modules["antenv.axon_hooks"] = mod
        antenv.axon_hooks = mod
        mod.set_axon_ntff_profile_hook(
            _ntff_profile_via_ctypes("/opt/axon/libaxon_pjrt.so"))
        return True
    except Exception as e:
        print(f"ntff hook wiring failed ({e}); running without trace")
        return False


def kernel(**inputs):
    from concourse.bass_utils import run_bass_kernel_spmd

    iters = int(os.environ.get("KERNEL_ITERS", ITERS))
    trace = bool(int(os.environ.get("KERNEL_TRACE", "0")))
    if trace:
        trace = _wire_ntff_hook()
    if iters not in _PROGRAM_CACHE:
        _PROGRAM_CACHE[iters] = _build_program(iters)
    nc = _PROGRAM_CACHE[iters]
    in_maps = _prep_inputs(**inputs)
    res = run_bass_kernel_spmd(nc, in_maps, core_ids=list(range(NC)), trace=trace)
    if trace and res.exec_time_ns is not None:
        print(f"HW exec time: {res.exec_time_ns} ns")
    rT_full = np.zeros((128, MP), np.float32)
    for c in range(NC):
        rT_full[:, c * R:(c + 1) * R] = res.results[c]["r_out"]
    r_fin = np.ascontiguousarray(rT_full.T)[:M0]
    out = _epilogue(r_fin, inputs["attractor_masses"], inputs["fw"], inputs["fb"])
    kernel.last_exec_time_ns = res.exec_time_ns if trace else None
    return out


if __name__ == "__main__":
    rng = np.random.default_rng(0)
    demo = {
        "x": rng.standard_normal((B, N, D)).astype(np.float32),
        "attractor_positions": rng.standard_normal((K, D)).astype(np.float32),
        "attractor_masses": np.full((K,), 0.1, np.float32),
        "init_velocities": np.zeros((K, D), np.float32),
        "pw1": rng.standard_normal((D, 2 * D)).astype(np.float32) / 16,
        "pb1": np.zeros((2 * D,), np.float32),
        "pw2": rng.standard_normal((2 * D, 1)).astype(np.float32) / 16,
        "pb2": np.zeros((1,), np.float32),
        "kw1": rng.standard_normal((D, 2 * D)).astype(np.float32) / 16,
        "kb1": np.zeros((2 * D,), np.float32),
        "kw2": rng.standard_normal((2 * D, 1)).astype(np.float32) / 16,
        "kb2": np.zeros((1,), np.float32),
        "bw1": rng.standard_normal((2 * D, HID)).astype(np.float32) / 16,
        "bb1": np.zeros((HID,), np.float32),
        "bw2": rng.standard_normal((HID, HID)).astype(np.float32) / 8,
        "bb2": np.zeros((HID,), np.float32),
        "bw3": rng.standard_normal((HID, D)).astype(np.float32) / 8,
        "bb3": np.zeros((D,), np.float32),
        "fw": rng.standard_normal((3 * D + 4, D)).astype(np.float32) / 20,
        "fb": np.zeros((D,), np.float32),
    }
    out = kernel(**demo)
    print("out", out.shape, np.abs(out).max())
